# revision 40
# baseline (speedup 1.0000x reference)
"""GPTQ-style grouped-dequant linear on 8 Trainium2 cores.

out[m,n] = sum_k A[m,k] * (q[n,k] - zeros[n,k//128]) * scales[n,k//128] + bias[n]
M=2048, K=4096, N=4096, group=128.

Sharding: column-parallel — qweight/scales/zeros/bias split along N (512/core),
A replicated.

Algebra: out = A @ (q*s)^T - rowsums_g(A) @ (z*s)^T + bias, where
rowsum_g[m] = sum_{k in group g} A[m,k]. The zeros/bias terms collapse into
ONE rank-33 matmul per output tile (lhsT = [rowsums; ones], rhs =
[-(z*s); bias]), so dequant is a single DVE multiply per k-group against a
DMA-partition-broadcast scales tile — no zero-point broadcast at all.

Host prep: A cast to bf16 (the kernel computes in bf16 regardless) and
transposed so the contraction dim lands on SBUF partitions; per-group A row
sums (same single pass over A); q repacked to uint8; small z/s/bias algebra.

Per core: scales rows are partition-broadcast by DMA from 16 DRAM replicas
(a single-row source serializes on one DRAM page), W^T tiles are produced
in [k,n] layout by one DVE multiply each, then one PSUM-accumulated bf16
matmul chain per 128-row output tile, opened by the rank-33 correction
matmul. Staggered lead chains keep the PE fed while W^T tiles stream.
"""

import numpy as np

import concourse.bass as bass
import concourse.mybir as mybir
import concourse.tile as tile
from concourse import bacc
from concourse.bass_utils import run_bass_kernel_spmd

P = 128
M, K, N = 2048, 4096, 4096
NCORES = 8
NS = N // NCORES          # 512 out-features per core
G = K // P                # 32 groups (group_size == P == 128)
MT = M // P               # 16 output row tiles
REP = 16                  # DRAM replicas of the scales rows

_cached = None


def _build():
    nc = bacc.Bacc("TRN2", target_bir_lowering=False, debug=False,
                   num_devices=NCORES)
    bf16, f32 = mybir.dt.bfloat16, mybir.dt.float32
    at = nc.dram_tensor("AT4", [MT, P, G, P], bf16, kind="ExternalInput")
    qt = nc.dram_tensor("q4", [P, G, NS], mybir.dt.uint8,
                        kind="ExternalInput")
    sr = nc.dram_tensor("srep", [REP, G, NS], bf16, kind="ExternalInput")
    # rsum and -(z*s)|bias packed side by side: one DMA loads both
    rz = nc.dram_tensor("rzpack", [G + 1, MT * P + NS], bf16,
                        kind="ExternalInput")
    out = nc.dram_tensor("out", [M, NS], f32, kind="ExternalOutput")

    with tile.TileContext(nc) as tc:
        with (
            tc.tile_pool(name="const", bufs=1) as const,
            tc.tile_pool(name="qpool", bufs=1) as qpool,
            tc.tile_pool(name="bcast", bufs=8) as bcp,
            tc.tile_pool(name="wt", bufs=1) as wtp,
            tc.tile_pool(name="apool", bufs=6) as apool,
            tc.tile_pool(name="mpsum", bufs=7, space="PSUM") as mpsum,
            tc.tile_pool(name="opool", bufs=3) as opool,
        ):
            # correction operands (tiny, land first on the Activation queue
            # so the SP queue starts immediately with lead A tiles)
            rzp = const.tile([G + 1, MT * P + NS], bf16, tag="rzp")
            nc.scalar.dma_start(out=rzp[:], in_=rz.ap()[:])
            rsum_sb = rzp[:, :MT * P].rearrange("p (mt j) -> p mt j", mt=MT)
            mzsb = rzp[:, MT * P:]

            # q^T as [p, g, n]: partition = k%128, one strip per k-group;
            # host layout makes each partition's span fully contiguous.
            # scales partition-broadcasts interleave with the q chunks on
            # the Activation queue, small chunks first so group 0's inputs
            # land with minimum latency.
            q8s = qpool.tile([P, G, NS], mybir.dt.uint8, tag="q8s")
            qr = qt.ap()
            s_tiles = {}

            def emit_sq(gq, eng):  # one q chunk + the matching paired s bcasts
                g0, g1 = gq
                eng.dma_start(out=q8s[:, g0:g1, :], in_=qr[:, g0:g1, :])
                for g in range(g0, g1, 2):
                    s_t = bcp.tile([P, 2, NS], bf16)
                    src = sr.ap()[:, g:g + 2, :].unsqueeze(1).to_broadcast(
                        (REP, P // REP, 2, NS))
                    eng.dma_start(out=s_t[:], in_=src)
                    s_tiles[g] = s_t[:, 0, :]
                    s_tiles[g + 1] = s_t[:, 1, :]

            # group 0/1 inputs ride the head of the SP queue (ahead of the
            # lead A tiles) so wt production starts immediately
            emit_sq((0, 2), nc.sync)
            for gq in [(2, 6), (6, 10), (10, 16), (16, 24), (24, 32)]:
                emit_sq(gq, nc.scalar)

            atr = at.ap()  # [MT, P, G, P], per-partition contiguous

            def load_ab(mt):
                # spread the A stream over all three DGE queues: leads 0-2 on
                # SP, leads 3-5 on the gpsimd SWDGE queue (stores only start
                # much later), phase-2 tiles on the Activation queue, which
                # drains its s/q work early and then carries nothing else
                if mt < NLEAD:
                    eng, nh = nc.sync, 4
                else:
                    # alternate phase-2 tiles between the two HW queues
                    eng, nh = (nc.scalar if mt % 2 else nc.sync), 4
                ab = apool.tile([P, G, P], bf16)
                for h in range(nh):
                    g0, g1 = h * (G // nh), (h + 1) * (G // nh)
                    eng.dma_start(out=ab[:, g0:g1, :],
                                  in_=atr[mt, :, g0:g1, :])
                return ab

            def open_chain(mt):
                # rank-33 correction matmul opens the PSUM accumulation:
                # psum = rowsums(A_mt) @ -(z*s) + 1 @ bias
                ps = mpsum.tile([P, NS], f32)
                nc.tensor.matmul(ps[:], rsum_sb[:, mt, :], mzsb[:],
                                 start=True, stop=False)
                return ps

            def finish(mt, ps):
                # copy on DVE: the Activation sequencer is a DMA-issue queue
                # here, and an in-order copy there would stall phase-2 A
                # prefetch behind each chain's PSUM drain
                ob = opool.tile([P, NS], f32)
                nc.vector.tensor_copy(ob[:], ps[:])
                # out stores ride the gpsimd SWDGE queue so they never
                # head-of-line-block the A-tile stream on the SP queue
                nc.gpsimd.dma_start(out=out.ap()[mt * P:(mt + 1) * P, :],
                                    in_=ob[:])

            # Phase 1: per k-group one DVE multiply produces the bf16 W^T
            # tile, consumed by NLEAD concurrently-open PSUM accumulation
            # chains. Leads join progressively (catch-up bursts) as their A
            # strips arrive.
            NLEAD = 7
            join_at = {0: 0, 1: 2, 2: 4, 3: 7, 4: 10, 5: 13, 6: 17}
            lead_ab = [load_ab(mt) for mt in range(NLEAD)]
            lead_ps = [open_chain(mt) for mt in range(NLEAD)]

            wts = []
            for g in range(G):
                wt = wtp.tile([P, NS], bf16, tag=f"wt{g}")
                nc.vector.tensor_tensor(wt[:], q8s[:, g, :], s_tiles[g],
                                        mybir.AluOpType.mult)
                wts.append(wt)
                for mt in range(NLEAD):
                    if join_at[mt] == g:
                        for gc in range(g + 1):  # catch-up burst
                            nc.tensor.matmul(lead_ps[mt][:],
                                             lead_ab[mt][:, gc, :], wts[gc][:],
                                             start=False,
                                             stop=(gc == G - 1))
                    elif join_at[mt] < g:
                        nc.tensor.matmul(lead_ps[mt][:], lead_ab[mt][:, g, :],
                                         wt[:], start=False,
                                         stop=(g == G - 1))
            for mt in range(NLEAD):
                finish(mt, lead_ps[mt])

            # Phase 2: remaining output tiles, dense back-to-back matmuls
            for mt in range(NLEAD, MT):
                ab = load_ab(mt)
                ps = open_chain(mt)
                for g in range(G):
                    nc.tensor.matmul(ps[:], ab[:, g, :], wts[g][:],
                                     start=False, stop=(g == G - 1))
                finish(mt, ps)

    nc.compile()
    return nc


def _prep_inputs(A, qweight, scales, zeros, bias):
    bf = mybir.dt.np(mybir.dt.bfloat16)
    # AT4[mt, p, g, j] = A[mt*128+j, g*128+p], cast to bf16 (the on-chip
    # pipeline computes the matmul in bf16 regardless)
    at4 = np.ascontiguousarray(
        A.reshape(MT, P, G, P).transpose(0, 3, 2, 1)).astype(bf)
    # per-group A row sums + ones row: rsum[g, mt, j] = sum_k A_g[mt*128+j]
    rsum = np.empty((G + 1, MT, P), dtype=np.float32)
    rsum[:G] = A.reshape(MT, P, G, P).sum(axis=3).transpose(2, 0, 1)
    rsum[G] = 1.0
    rsum = rsum.astype(bf)
    in_maps = []
    for c in range(NCORES):
        r = slice(c * NS, (c + 1) * NS)
        # q4[p, g, n] = q[n, g*128+p]
        q4 = np.ascontiguousarray(
            qweight[r].astype(np.uint8).T.reshape(G, P, NS).transpose(1, 0, 2))
        sT = scales[r].T.astype(bf)                      # [G, NS]
        rzpack = np.empty((G + 1, MT * P + NS), dtype=np.float32)
        rzpack[:, :MT * P] = rsum.reshape(G + 1, MT * P)
        rzpack[:G, MT * P:] = -(zeros[r] * scales[r]).T  # -(z*s)
        rzpack[G, MT * P:] = bias[r]
        in_maps.append({
            "AT4": at4,
            "q4": q4,
            "srep": np.ascontiguousarray(np.broadcast_to(sT, (REP, G, NS))),
            "rzpack": rzpack.astype(bf),
        })
    return in_maps


def run(inputs, **spmd_kwargs):
    global _cached
    if _cached is None:
        _cached = _build()
    in_maps = _prep_inputs(**inputs)
    res = run_bass_kernel_spmd(_cached, in_maps, list(range(NCORES)),
                               **spmd_kwargs)
    outp = np.concatenate([res.results[c]["out"] for c in range(NCORES)],
                          axis=1)
    return outp, res


def kernel(**inputs):
    return run(inputs)[0]


# revision 42
# speedup vs baseline: 1.0532x; 1.0532x over previous
"""GPTQ-style grouped-dequant linear on 8 Trainium2 cores.

out[m,n] = sum_k A[m,k] * (q[n,k] - zeros[n,k//128]) * scales[n,k//128] + bias[n]
M=2048, K=4096, N=4096, group=128.

Sharding: column-parallel — qweight/scales/zeros/bias split along N (512/core),
A replicated.

Algebra: out = A @ (q*s)^T - rowsums_g(A) @ (z*s)^T + bias, where
rowsum_g[m] = sum_{k in group g} A[m,k]. The zeros/bias terms collapse into
ONE rank-33 matmul per output tile (lhsT = [rowsums; ones], rhs =
[-(z*s); bias]), so dequant is a single DVE multiply per k-group against a
DMA-partition-broadcast scales tile — no zero-point broadcast at all.

Host prep: A cast to bf16 (the kernel computes in bf16 regardless) and
transposed so the contraction dim lands on SBUF partitions; per-group A row
sums (same single pass over A); q repacked to uint8; small z/s/bias algebra.

Per core: scales rows are partition-broadcast by DMA from 16 DRAM replicas
(a single-row source serializes on one DRAM page), W^T tiles are produced
in [k,n] layout by one DVE multiply each, then one PSUM-accumulated bf16
matmul chain per 128-row output tile, opened by the rank-33 correction
matmul. Staggered lead chains keep the PE fed while W^T tiles stream.
"""

import numpy as np

import concourse.bass as bass
import concourse.mybir as mybir
import concourse.tile as tile
from concourse import bacc
from concourse.bass_utils import run_bass_kernel_spmd

P = 128
M, K, N = 2048, 4096, 4096
NCORES = 8
NS = N // NCORES          # 512 out-features per core
G = K // P                # 32 groups (group_size == P == 128)
MT = M // P               # 16 output row tiles
REP = 16                  # DRAM replicas of the scales rows

_cached = None


def _build():
    nc = bacc.Bacc("TRN2", target_bir_lowering=False, debug=False,
                   num_devices=NCORES)
    bf16, f32 = mybir.dt.bfloat16, mybir.dt.float32
    at = nc.dram_tensor("AT4", [MT, P, G, P], bf16, kind="ExternalInput")
    qt = nc.dram_tensor("q4", [P, G, NS], mybir.dt.uint8,
                        kind="ExternalInput")
    sr = nc.dram_tensor("srep", [REP, G, NS], bf16, kind="ExternalInput")
    # rsum and -(z*s)|bias packed side by side: one DMA loads both
    rz = nc.dram_tensor("rzpack", [G + 1, MT * P + NS], bf16,
                        kind="ExternalInput")
    out = nc.dram_tensor("out", [M, NS], f32, kind="ExternalOutput")

    with tile.TileContext(nc) as tc:
        with (
            tc.tile_pool(name="const", bufs=1) as const,
            tc.tile_pool(name="qpool", bufs=1) as qpool,
            tc.tile_pool(name="bcast", bufs=8) as bcp,
            tc.tile_pool(name="wt", bufs=1) as wtp,
            tc.tile_pool(name="apool", bufs=6) as apool,
            tc.tile_pool(name="mpsum", bufs=7, space="PSUM") as mpsum,
            tc.tile_pool(name="opool", bufs=3) as opool,
        ):
            # correction operands (tiny, land first on the Activation queue
            # so the SP queue starts immediately with lead A tiles)
            rzp = const.tile([G + 1, MT * P + NS], bf16, tag="rzp")
            nc.scalar.dma_start(out=rzp[:], in_=rz.ap()[:])
            rsum_sb = rzp[:, :MT * P].rearrange("p (mt j) -> p mt j", mt=MT)
            mzsb = rzp[:, MT * P:]

            # q^T as [p, g, n]: partition = k%128, one strip per k-group;
            # host layout makes each partition's span fully contiguous.
            # scales partition-broadcasts interleave with the q chunks on
            # the Activation queue, small chunks first so group 0's inputs
            # land with minimum latency.
            q8s = qpool.tile([P, G, NS], mybir.dt.uint8, tag="q8s")
            qr = qt.ap()
            s_tiles = {}

            def emit_sq(gq, eng):  # one q chunk + the matching paired s bcasts
                g0, g1 = gq
                eng.dma_start(out=q8s[:, g0:g1, :], in_=qr[:, g0:g1, :])
                for g in range(g0, g1, 2):
                    s_t = bcp.tile([P, 2, NS], bf16)
                    src = sr.ap()[:, g:g + 2, :].unsqueeze(1).to_broadcast(
                        (REP, P // REP, 2, NS))
                    eng.dma_start(out=s_t[:], in_=src)
                    s_tiles[g] = s_t[:, 0, :]
                    s_tiles[g + 1] = s_t[:, 1, :]

            for gq in [(0, 2), (2, 6), (6, 10), (10, 16), (16, 24), (24, 32)]:
                emit_sq(gq, nc.scalar)

            atr = at.ap()  # [MT, P, G, P], per-partition contiguous

            def load_ab(mt):
                # spread the A stream over all three DGE queues: leads 0-2 on
                # SP, leads 3-5 on the gpsimd SWDGE queue (stores only start
                # much later), phase-2 tiles on the Activation queue, which
                # drains its s/q work early and then carries nothing else
                if mt < NLEAD:
                    eng, nh = nc.sync, 4
                else:
                    # alternate phase-2 tiles between the two HW queues
                    eng, nh = (nc.scalar if mt % 2 else nc.sync), 4
                ab = apool.tile([P, G, P], bf16)
                for h in range(nh):
                    g0, g1 = h * (G // nh), (h + 1) * (G // nh)
                    eng.dma_start(out=ab[:, g0:g1, :],
                                  in_=atr[mt, :, g0:g1, :])
                return ab

            def open_chain(mt):
                # rank-33 correction matmul opens the PSUM accumulation:
                # psum = rowsums(A_mt) @ -(z*s) + 1 @ bias
                ps = mpsum.tile([P, NS], f32)
                nc.tensor.matmul(ps[:], rsum_sb[:, mt, :], mzsb[:],
                                 start=True, stop=False)
                return ps

            def finish(mt, ps):
                # copy on DVE: the Activation sequencer is a DMA-issue queue
                # here, and an in-order copy there would stall phase-2 A
                # prefetch behind each chain's PSUM drain
                ob = opool.tile([P, NS], f32)
                nc.vector.tensor_copy(ob[:], ps[:])
                # out stores ride the gpsimd SWDGE queue so they never
                # head-of-line-block the A-tile stream on the SP queue
                nc.gpsimd.dma_start(out=out.ap()[mt * P:(mt + 1) * P, :],
                                    in_=ob[:])

            # Phase 1: per k-group one DVE multiply produces the bf16 W^T
            # tile, consumed by NLEAD concurrently-open PSUM accumulation
            # chains. Leads join progressively (catch-up bursts) as their A
            # strips arrive.
            NLEAD = 6
            join_at = {0: 0, 1: 2, 2: 5, 3: 8, 4: 12, 5: 16}
            lead_ab = [load_ab(mt) for mt in range(NLEAD)]
            lead_ps = [open_chain(mt) for mt in range(NLEAD)]

            wts = []
            for g in range(G):
                wt = wtp.tile([P, NS], bf16, tag=f"wt{g}")
                nc.vector.tensor_tensor(wt[:], q8s[:, g, :], s_tiles[g],
                                        mybir.AluOpType.mult)
                wts.append(wt)
                for mt in range(NLEAD):
                    if join_at[mt] == g:
                        for gc in range(g + 1):  # catch-up burst
                            nc.tensor.matmul(lead_ps[mt][:],
                                             lead_ab[mt][:, gc, :], wts[gc][:],
                                             start=False,
                                             stop=(gc == G - 1))
                    elif join_at[mt] < g:
                        nc.tensor.matmul(lead_ps[mt][:], lead_ab[mt][:, g, :],
                                         wt[:], start=False,
                                         stop=(g == G - 1))
            for mt in range(NLEAD):
                finish(mt, lead_ps[mt])

            # Phase 2: remaining output tiles, dense back-to-back matmuls
            for mt in range(NLEAD, MT):
                ab = load_ab(mt)
                ps = open_chain(mt)
                for g in range(G):
                    nc.tensor.matmul(ps[:], ab[:, g, :], wts[g][:],
                                     start=False, stop=(g == G - 1))
                finish(mt, ps)

    nc.compile()
    return nc


def _prep_inputs(A, qweight, scales, zeros, bias):
    bf = mybir.dt.np(mybir.dt.bfloat16)
    # AT4[mt, p, g, j] = A[mt*128+j, g*128+p], cast to bf16 (the on-chip
    # pipeline computes the matmul in bf16 regardless)
    at4 = np.ascontiguousarray(
        A.reshape(MT, P, G, P).transpose(0, 3, 2, 1)).astype(bf)
    # per-group A row sums + ones row: rsum[g, mt, j] = sum_k A_g[mt*128+j]
    rsum = np.empty((G + 1, MT, P), dtype=np.float32)
    rsum[:G] = A.reshape(MT, P, G, P).sum(axis=3).transpose(2, 0, 1)
    rsum[G] = 1.0
    rsum = rsum.astype(bf)
    in_maps = []
    for c in range(NCORES):
        r = slice(c * NS, (c + 1) * NS)
        # q4[p, g, n] = q[n, g*128+p]
        q4 = np.ascontiguousarray(
            qweight[r].astype(np.uint8).T.reshape(G, P, NS).transpose(1, 0, 2))
        sT = scales[r].T.astype(bf)                      # [G, NS]
        rzpack = np.empty((G + 1, MT * P + NS), dtype=np.float32)
        rzpack[:, :MT * P] = rsum.reshape(G + 1, MT * P)
        rzpack[:G, MT * P:] = -(zeros[r] * scales[r]).T  # -(z*s)
        rzpack[G, MT * P:] = bias[r]
        in_maps.append({
            "AT4": at4,
            "q4": q4,
            "srep": np.ascontiguousarray(np.broadcast_to(sT, (REP, G, NS))),
            "rzpack": rzpack.astype(bf),
        })
    return in_maps


def run(inputs, **spmd_kwargs):
    global _cached
    if _cached is None:
        _cached = _build()
    in_maps = _prep_inputs(**inputs)
    res = run_bass_kernel_spmd(_cached, in_maps, list(range(NCORES)),
                               **spmd_kwargs)
    outp = np.concatenate([res.results[c]["out"] for c in range(NCORES)],
                          axis=1)
    return outp, res


def kernel(**inputs):
    return run(inputs)[0]
